# revision 12
# baseline (speedup 1.0000x reference)
"""Binarized 3x3 conv (N=32, C=256->256, H=W=56, pad 1) on 8 TRN2 NeuronCores.

Sharding: data-parallel over batch (4 images per core), weights replicated.

Math: binarize exactly via
  xb = (x >= 0) - 0.5            in {+-0.5}  (exact in fp8 e4m3)
  wb = 4*(w >= 0) - 2            in {+-2}    (exact in fp8 e4m3)
so every product is exactly +-1 and fp32 PSUM accumulation is exact
(integer partial sums, |.| <= 2304 << 2^24). sign(0)=+1 is honored.

Conv as matmul: the padded (58x58) binarized image lives flat in SBUF, so for
each kernel tap (kh,kw) the needed input window is a CONTIGUOUS span of the
flat padded grid shifted by (kh-1)*58+(kw-1). Outputs are computed on the
padded grid (464-wide spans = 8 padded rows) and the two garbage columns per
row (conv centered on pad columns) are dropped at drain time.

TensorE: fp8 DoubleRow matmuls contract all 256 input channels in one
instruction (K=128 partitions x 2 interleaved weights/cell), 9 accumulating
matmuls (one per tap) per output tile. 2 co-chunks x 4 images x 7 row-groups
x 9 taps = 504 matmuls per core.

Weights: strided gather-DMA loads w[co,ci,kh,kw] directly into a
[ci_local][co][two][tap] staging layout (36B descriptor runs), then one DVE
binarize+permute writes the DoubleRow layout [tap][two][co]. No transposes.
"""

import numpy as np

import concourse.bass as bass
import concourse.mybir as mybir
import concourse.tile as tile
from concourse import bacc, bass_utils

N_CORES = 8
N, CIN, H, W = 32, 256, 56, 56
COUT, KS = 256, 3
NPC = N // N_CORES          # images per core
HP, WP = H + 2, W + 2       # padded spatial (58x58)
GRID = HP * WP              # 3364
LEAD = 64                   # per-chunk front pad so tap offsets never go negative
CHUNK = 3440                # LEAD + GRID + 12 tail, %16 == 0 (DoubleRow step)
NROW_GROUPS = 7
ROWS_PER_GROUP = H // NROW_GROUPS   # 8
FREE = ROWS_PER_GROUP * WP          # 464 <= 512 (one PSUM bank, fp32)
OFREE = ROWS_PER_GROUP * W          # 448 valid output columns
CI_CHUNKS = CIN // 128
CO_CHUNKS = COUT // 128

F32 = mybir.dt.float32
FP8 = mybir.dt.float8e4
ALU = mybir.AluOpType
AF = mybir.ActivationFunctionType
DR = mybir.MatmulPerfMode.DoubleRow


def _body(tc, x_d, w_d, b_d, o_d, repeats=1, parts='full'):
    nc = tc.nc

    from contextlib import ExitStack
    ctx = ExitStack()
    with ctx:
        const_pool = ctx.enter_context(tc.tile_pool(name="const", bufs=1))
        wd_pool = ctx.enter_context(tc.tile_pool(name="wd", bufs=1))
        xpad_pool = ctx.enter_context(tc.tile_pool(name="xpad", bufs=1))
        xin_pool = ctx.enter_context(tc.tile_pool(name="xin", bufs=2))
        out_pool = ctx.enter_context(tc.tile_pool(name="outs", bufs=2))

        bias_sb = const_pool.tile([128, CO_CHUNKS], F32, tag="bias", name="bias_sb")
        nc.sync.dma_start(bias_sb[:], b_d.rearrange("(c p) -> p c", p=128))

        # ---- weight phase ----
        # wd8[cc]: [128 ci_local, 9*256] fp8, free idx = tap*256 + two*128 + co
        # (lhsT slice per tap: [k][two][m], steps [128, 1] - verified DoubleRow
        # pairing: contraction pairs (k, two) elementwise on both operands)
        wd8 = []
        with tc.tile_pool(name="wstage", bufs=2) as wstage:
            for cc in range(CO_CHUNKS):
                wstg = wstage.tile([128, 2304], F32, tag="wstg", name=f"wstg{cc}")
                # gather: dst [k][co][two][tap] contiguous; src strides
                # (ci 9, co 2304, two 1152, tap 1) - 36B runs
                src = w_d[cc * 128:(cc + 1) * 128] \
                    .rearrange("m (two k) kh kw -> k m two (kh kw)", two=2)
                dst = wstg[:].rearrange("k (m two kp) -> k m two kp",
                                        two=2, kp=KS * KS)
                nc.sync.dma_start(dst, src)
                wt = wd_pool.tile([128, KS * KS * 256], FP8, tag=f"wd{cc}",
                                  name=f"wd8_{cc}")
                wd8.append(wt)
                # {4,0} permuted into DoubleRow layout, then -2 -> {+2,-2}
                nc.vector.tensor_scalar(
                    wt[:].rearrange("k (kp two m) -> k kp two m",
                                    two=2, kp=KS * KS),
                    wstg[:].rearrange("k (m two kp) -> k kp two m",
                                      two=2, kp=KS * KS),
                    0.0, 4.0, op0=ALU.is_ge, op1=ALU.mult)
                nc.vector.tensor_scalar(wt[:], wt[:], 2.0, None,
                                        op0=ALU.subtract)

        # ---- input + conv phases (repeated `repeats` times for benching) ----
        with tc.tile_pool(name="cpsum", bufs=1, space="PSUM") as cpsum:
            o_d3 = [[o_d[n, cc * 128:(cc + 1) * 128].rearrange("c h w -> c (h w)")
                     for cc in range(CO_CHUNKS)] for n in range(NPC)]
            for rep in range(repeats):
                xr = []
                for two in range(CI_CHUNKS):
                    x_raw = xin_pool.tile([128, NPC * H * W], F32, tag="xraw",
                                          name=f"xraw{rep}_{two}")
                    xr.append(x_raw)
                    nc.sync.dma_start(
                        x_raw[:].rearrange("c (n s) -> c n s", n=NPC),
                        x_d[:, two * 128:(two + 1) * 128]
                        .rearrange("n c h w -> c n (h w)"))
                xp = []
                for n in range(NPC):
                    t = xpad_pool.tile([128, 2 * CHUNK], FP8,
                                       tag=f"xp{n}", name=f"xp{rep}_{n}")
                    xp.append(t)
                    for two in range(CI_CHUNKS):
                        nc.gpsimd.memset(
                            t[:, two * CHUNK:two * CHUNK + LEAD], 0.0)
                        base = two * CHUNK + LEAD
                        g = t[:, base:base + GRID] \
                            .rearrange("c (h w) -> c h w", w=WP)
                        nc.gpsimd.memset(g[:, 0:1, :], 0.0)
                        nc.gpsimd.memset(g[:, HP - 1:HP, :], 0.0)
                        nc.gpsimd.memset(g[:, 1:HP - 1, 0:1], 0.0)
                        nc.gpsimd.memset(g[:, 1:HP - 1, WP - 1:WP], 0.0)
                        nc.gpsimd.memset(
                            t[:, base + GRID:(two + 1) * CHUNK], 0.0)
                        nc.vector.tensor_scalar(
                            g[:, 1:H + 1, 1:W + 1],
                            xr[two][:, n * H * W:(n + 1) * H * W]
                            .rearrange("c (h w) -> c h w", w=W),
                            0.0, 0.5, op0=ALU.is_ge, op1=ALU.subtract)

                for cc in range(CO_CHUNKS if parts != 'input' else 0):
                    for n in range(NPC):
                        pp = cpsum.tile([128, NROW_GROUPS * 512], F32,
                                        tag="cps", name=f"cps{rep}_{cc}_{n}",
                                        bufs=1)
                        for kpos in range(KS * KS):
                            kh, kw = divmod(kpos, KS)
                            lhsT = wd8[cc][:, kpos * 256:(kpos + 1) * 256] \
                                .rearrange("k (two m) -> k two m", two=2)
                            for rg in range(NROW_GROUPS):
                                off = (LEAD + WP + rg * FREE
                                       + (kh - 1) * WP + (kw - 1))
                                rhs = xp[n][:].rearrange(
                                    "k (two s) -> k two s",
                                    s=CHUNK)[:, :, off:off + FREE]
                                nc.tensor.matmul(
                                    pp[:, rg * 512:rg * 512 + FREE], lhsT, rhs,
                                    start=(kpos == 0), stop=(kpos == KS * KS - 1),
                                    perf_mode=DR)
                        ob = out_pool.tile([128, NROW_GROUPS * OFREE], F32,
                                           tag="ob", name=f"ob{rep}_{cc}_{n}")
                        drain_in = pp[:].rearrange(
                            "m (g s) -> m g s", g=NROW_GROUPS)[:, :, :FREE] \
                            .rearrange("m g (r c) -> m g r c", c=WP
                                       )[:, :, :, 1:W + 1]
                        drain_out = ob[:].rearrange(
                            "m (g r c) -> m g r c", g=NROW_GROUPS, c=W)
                        nc.scalar.activation(
                            drain_out, drain_in,
                            AF.Identity, bias=bias_sb[:, cc:cc + 1],
                            scale=1.0)
                        if parts != 'nooutdma':
                            nc.sync.dma_start(o_d3[n][cc], ob[:])


_nc_cache = {}


def _get_nc(repeats=1, parts='full'):
    key = (repeats, parts)
    if key not in _nc_cache:
        nc = bacc.Bacc("TRN2", debug=False)
        x_d = nc.dram_tensor("x", [NPC, CIN, H, W], F32, kind="ExternalInput").ap()
        w_d = nc.dram_tensor("w", [COUT, CIN, KS, KS], F32,
                             kind="ExternalInput").ap()
        b_d = nc.dram_tensor("b", [COUT], F32, kind="ExternalInput").ap()
        o_d = nc.dram_tensor("out", [NPC, COUT, H, W], F32,
                             kind="ExternalOutput").ap()
        with tile.TileContext(nc) as tc:
            _body(tc, x_d, w_d, b_d, o_d, repeats=repeats, parts=parts)
        nc.compile()
        _nc_cache[key] = nc
    return _nc_cache[key]


def _run(inputs, repeats=1, **kwargs):
    x, w, b = inputs["x"], inputs["w"], inputs["b"]
    assert x.shape == (N, CIN, H, W), x.shape
    nc = _get_nc(repeats)
    in_maps = [{
        "x": np.ascontiguousarray(x[i * NPC:(i + 1) * NPC], dtype=np.float32),
        "w": np.ascontiguousarray(w, dtype=np.float32),
        "b": np.ascontiguousarray(b, dtype=np.float32),
    } for i in range(N_CORES)]
    res = bass_utils.run_bass_kernel_spmd(
        nc, in_maps, core_ids=list(range(N_CORES)), **kwargs)
    out = np.concatenate([res.results[i]["out"] for i in range(N_CORES)], axis=0)
    return out, res


def kernel(**inputs) -> np.ndarray:
    out, _ = _run(inputs)
    return out


# revision 13
# speedup vs baseline: 1.5004x; 1.5004x over previous
"""Binarized 3x3 conv (N=32, C=256->256, H=W=56, pad 1) on 8 TRN2 NeuronCores.

Sharding: data-parallel over batch (4 images per core), weights replicated.

Math: binarize exactly via
  xb = (x >= 0) - 0.5            in {+-0.5}  (exact in fp8 e4m3)
  wb = 4*(w >= 0) - 2            in {+-2}    (exact in fp8 e4m3)
so every product is exactly +-1 and fp32 PSUM accumulation is exact
(integer partial sums, |.| <= 2304 << 2^24). sign(0)=+1 is honored.

Conv as matmul: the padded (58x58) binarized image lives flat in SBUF, so for
each kernel tap (kh,kw) the needed input window is a CONTIGUOUS span of the
flat padded grid shifted by (kh-1)*58+(kw-1). Outputs are computed on the
padded grid (464-wide spans = 8 padded rows) and the two garbage columns per
row (conv centered on pad columns) are dropped at drain time.

TensorE: fp8 DoubleRow matmuls contract all 256 input channels in one
instruction (K=128 partitions x 2 interleaved weights/cell), 9 accumulating
matmuls (one per tap) per output tile. 2 co-chunks x 4 images x 7 row-groups
x 9 taps = 504 matmuls per core.

Weights: strided gather-DMA loads w[co,ci,kh,kw] directly into a
[ci_local][co][two][tap] staging layout (36B descriptor runs), then one DVE
binarize+permute writes the DoubleRow layout [tap][two][co]. No transposes.
"""

import os
os.environ.setdefault("CONCOURSE_SCRUB_NEFF_DEBUG_INFO", "1")

import numpy as np

import concourse.bass as bass
import concourse.mybir as mybir
import concourse.tile as tile
from concourse import bacc, bass_utils

N_CORES = 8
N, CIN, H, W = 32, 256, 56, 56
COUT, KS = 256, 3
NPC = N // N_CORES          # images per core
HP, WP = H + 2, W + 2       # padded spatial (58x58)
GRID = HP * WP              # 3364
LEAD = 64                   # per-chunk front pad so tap offsets never go negative
CHUNK = 3440                # LEAD + GRID + 12 tail, %16 == 0 (DoubleRow step)
NROW_GROUPS = 7
ROWS_PER_GROUP = H // NROW_GROUPS   # 8
FREE = ROWS_PER_GROUP * WP          # 464 <= 512 (one PSUM bank, fp32)
OFREE = ROWS_PER_GROUP * W          # 448 valid output columns
CI_CHUNKS = CIN // 128
CO_CHUNKS = COUT // 128

F32 = mybir.dt.float32
FP8 = mybir.dt.float8e4
ALU = mybir.AluOpType
AF = mybir.ActivationFunctionType
DR = mybir.MatmulPerfMode.DoubleRow


def _body(tc, x_d, w_d, b_d, o_d, repeats=1, parts='full'):
    nc = tc.nc

    from contextlib import ExitStack
    ctx = ExitStack()
    with ctx:
        const_pool = ctx.enter_context(tc.tile_pool(name="const", bufs=1))
        wd_pool = ctx.enter_context(tc.tile_pool(name="wd", bufs=1))
        xpad_pool = ctx.enter_context(tc.tile_pool(name="xpad", bufs=1))
        xin_pool = ctx.enter_context(tc.tile_pool(name="xin", bufs=2))
        out_pool = ctx.enter_context(tc.tile_pool(name="outs", bufs=2))

        bias_sb = const_pool.tile([128, CO_CHUNKS], F32, tag="bias", name="bias_sb")
        nc.sync.dma_start(bias_sb[:], b_d.rearrange("(c p) -> p c", p=128))

        # ---- weight phase ----
        # wd8[cc]: [128 ci_local, 9*256] fp8, free idx = tap*256 + two*128 + co
        # (lhsT slice per tap: [k][two][m], steps [128, 1] - verified DoubleRow
        # pairing: contraction pairs (k, two) elementwise on both operands)
        wd8 = []
        with tc.tile_pool(name="wstage", bufs=2) as wstage:
            for cc in range(CO_CHUNKS):
                wstg = wstage.tile([128, 2304], F32, tag="wstg", name=f"wstg{cc}")
                # gather: dst [k][co][two][tap] contiguous; src strides
                # (ci 9, co 2304, two 1152, tap 1) - 36B runs
                src = w_d[cc * 128:(cc + 1) * 128] \
                    .rearrange("m (two k) kh kw -> k m two (kh kw)", two=2)
                dst = wstg[:].rearrange("k (m two kp) -> k m two kp",
                                        two=2, kp=KS * KS)
                nc.sync.dma_start(dst, src)
                wt = wd_pool.tile([128, KS * KS * 256], FP8, tag=f"wd{cc}",
                                  name=f"wd8_{cc}")
                wd8.append(wt)
                # {4,0} permuted into DoubleRow layout, then -2 -> {+2,-2}
                nc.vector.tensor_scalar(
                    wt[:].rearrange("k (kp two m) -> k kp two m",
                                    two=2, kp=KS * KS),
                    wstg[:].rearrange("k (m two kp) -> k kp two m",
                                      two=2, kp=KS * KS),
                    0.0, 4.0, op0=ALU.is_ge, op1=ALU.mult)
                nc.vector.tensor_scalar(wt[:], wt[:], 2.0, None,
                                        op0=ALU.subtract)

        # ---- input + conv phases (repeated `repeats` times for benching) ----
        with tc.tile_pool(name="cpsum", bufs=1, space="PSUM") as cpsum:
            o_d3 = [[o_d[n, cc * 128:(cc + 1) * 128].rearrange("c h w -> c (h w)")
                     for cc in range(CO_CHUNKS)] for n in range(NPC)]
            for rep in range(repeats):
                xr = []
                for two in range(CI_CHUNKS):
                    x_raw = xin_pool.tile([128, NPC * H * W], F32, tag="xraw",
                                          name=f"xraw{rep}_{two}")
                    xr.append(x_raw)
                    nc.sync.dma_start(
                        x_raw[:].rearrange("c (n s) -> c n s", n=NPC),
                        x_d[:, two * 128:(two + 1) * 128]
                        .rearrange("n c h w -> c n (h w)"))
                # one tensor holds all 8 (image, ci-chunk) padded grids;
                # borders zeroed with 6 multi-grid strided memsets, binarize
                # is 2 ops (one per ci-chunk, all 4 images at once)
                xpall = xpad_pool.tile([128, NPC * 2 * CHUNK], FP8,
                                       tag="xpall", name=f"xpall{rep}")
                xg = xpall[:].rearrange("c (g s) -> c g s", s=CHUNK)
                nc.gpsimd.memset(xg[:, :, 0:LEAD], 0.0)
                nc.gpsimd.memset(xg[:, :, LEAD + GRID:CHUNK], 0.0)
                xgrid = xg[:, :, LEAD:LEAD + GRID] \
                    .rearrange("c g (h w) -> c g h w", w=WP)
                nc.gpsimd.memset(xgrid[:, :, 0:1, :], 0.0)
                nc.gpsimd.memset(xgrid[:, :, HP - 1:HP, :], 0.0)
                nc.gpsimd.memset(xgrid[:, :, 1:HP - 1, 0:1], 0.0)
                nc.gpsimd.memset(xgrid[:, :, 1:HP - 1, WP - 1:WP], 0.0)
                for two in range(CI_CHUNKS):
                    dst = xpall[:, two * CHUNK:] if False else None
                    nc.vector.tensor_scalar(
                        xpall[:].rearrange("c (n t s) -> c n t s",
                                           t=2, s=CHUNK)[:, :, two,
                                                         LEAD:LEAD + GRID]
                        .rearrange("c n (h w) -> c n h w", w=WP
                                   )[:, :, 1:H + 1, 1:W + 1],
                        xr[two][:].rearrange("c (n h w) -> c n h w",
                                             n=NPC, w=W),
                        0.0, 0.5, op0=ALU.is_ge, op1=ALU.subtract)
                xp = [xpall[:, n * 2 * CHUNK:(n + 1) * 2 * CHUNK]
                      for n in range(NPC)]

                for cc in range(CO_CHUNKS if parts != 'input' else 0):
                    for n in range(NPC):
                        pp = cpsum.tile([128, NROW_GROUPS * 512], F32,
                                        tag="cps", name=f"cps{rep}_{cc}_{n}",
                                        bufs=1)
                        for kpos in range(KS * KS):
                            kh, kw = divmod(kpos, KS)
                            lhsT = wd8[cc][:, kpos * 256:(kpos + 1) * 256] \
                                .rearrange("k (two m) -> k two m", two=2)
                            for rg in range(NROW_GROUPS):
                                off = (LEAD + WP + rg * FREE
                                       + (kh - 1) * WP + (kw - 1))
                                rhs = xp[n].rearrange(
                                    "k (two s) -> k two s",
                                    s=CHUNK)[:, :, off:off + FREE]
                                nc.tensor.matmul(
                                    pp[:, rg * 512:rg * 512 + FREE], lhsT, rhs,
                                    start=(kpos == 0), stop=(kpos == KS * KS - 1),
                                    perf_mode=DR)
                        ob = out_pool.tile([128, NROW_GROUPS * OFREE], F32,
                                           tag="ob", name=f"ob{rep}_{cc}_{n}")
                        drain_in = pp[:].rearrange(
                            "m (g s) -> m g s", g=NROW_GROUPS)[:, :, :FREE] \
                            .rearrange("m g (r c) -> m g r c", c=WP
                                       )[:, :, :, 1:W + 1]
                        drain_out = ob[:].rearrange(
                            "m (g r c) -> m g r c", g=NROW_GROUPS, c=W)
                        nc.scalar.activation(
                            drain_out, drain_in,
                            AF.Identity, bias=bias_sb[:, cc:cc + 1],
                            scale=1.0)
                        if parts != 'nooutdma':
                            nc.sync.dma_start(o_d3[n][cc], ob[:])


_nc_cache = {}


def _get_nc(repeats=1, parts='full'):
    key = (repeats, parts)
    if key not in _nc_cache:
        nc = bacc.Bacc("TRN2", debug=False)
        x_d = nc.dram_tensor("x", [NPC, CIN, H, W], F32, kind="ExternalInput").ap()
        w_d = nc.dram_tensor("w", [COUT, CIN, KS, KS], F32,
                             kind="ExternalInput").ap()
        b_d = nc.dram_tensor("b", [COUT], F32, kind="ExternalInput").ap()
        o_d = nc.dram_tensor("out", [NPC, COUT, H, W], F32,
                             kind="ExternalOutput").ap()
        with tile.TileContext(nc) as tc:
            _body(tc, x_d, w_d, b_d, o_d, repeats=repeats, parts=parts)
        nc.compile()
        _nc_cache[key] = nc
    return _nc_cache[key]


def _run(inputs, repeats=1, **kwargs):
    x, w, b = inputs["x"], inputs["w"], inputs["b"]
    assert x.shape == (N, CIN, H, W), x.shape
    nc = _get_nc(repeats)
    in_maps = [{
        "x": np.ascontiguousarray(x[i * NPC:(i + 1) * NPC], dtype=np.float32),
        "w": np.ascontiguousarray(w, dtype=np.float32),
        "b": np.ascontiguousarray(b, dtype=np.float32),
    } for i in range(N_CORES)]
    res = bass_utils.run_bass_kernel_spmd(
        nc, in_maps, core_ids=list(range(N_CORES)), **kwargs)
    out = np.concatenate([res.results[i]["out"] for i in range(N_CORES)], axis=0)
    return out, res


def kernel(**inputs) -> np.ndarray:
    out, _ = _run(inputs)
    return out
